# revision 9
# baseline (speedup 1.0000x reference)
"""MoE top-2 Trainium2 kernel — 4-way expert-F-split for load balance.

Experts are split into two teams of four (snake assignment on 16-aligned
token counts, so both teams' rank-sorted capacity lists nearly match).
Each core hosts one F-quarter of all four experts of its team; per-core
columns = sum of the four slot capacities (2080 here vs 2-way's 1056 at
double the per-column work): PE cycles = 128 * sum(caps) ~= 266k.

Each slot is chunked into the fewest <=512-col pieces (272x2, 264x2,
512, 496 here), keeping the matmul count at the 2-way level so
per-instruction overhead doesn't eat the cycle savings. One shared PSUM
pool with a per-shape tag (2 bufs each) fits the four shapes in exactly
8 banks.

v2 scheduling changes (from ntff trace analysis):
- dma_start count cut 79 -> 35 (w1 slots 1-3 and w2 are loaded in big
  merged transfers; slot-0 w1 and x stay fine-grained for the head).
  The end-of-kernel barrier expands into a per-engine EVENT_SEMAPHORE
  ladder whose length tracks the number of DMA semaphores — fewer
  dma_starts => shorter epilogue (was ~8us on the PE queue).
- warmup matmuls 112 -> 64: just enough to cover the HAM cold window
  and the first w1/x tile DMA; the old size delayed the first real
  matmul by ~3us.
- the last d-iteration's y store is split per chunk so the final DMA
  after the last matmul is ~127KB instead of 532KB.

Device output is the coef-weighted PARTIAL y (per F-quarter) in bf16;
the host sums each expert's four partials and scatters.
"""

import numpy as np
import ml_dtypes

import sys

if "/opt/trn_rl_repo" not in sys.path:
    sys.path.insert(0, "/opt/trn_rl_repo")

import concourse.tile as tile
from concourse import bacc, mybir
from concourse.bass_utils import run_bass_kernel_spmd

BF16 = ml_dtypes.bfloat16

E, D, F, NTOK = 8, 1024, 4096, 2048
P = 128
KD = D // P          # 8 k-tiles over D
NS = 4               # experts (slots) per core
FQ = F // NS         # 1024: F-quarter per expert per core
KFL = FQ // P        # 8 f-tiles per slot
NCORE = 8
N_WARM = 104

_NC_CACHE: dict = {}


def _build_nc(caps: tuple):
    assert len(caps) == NS
    Ls = []
    nchs = []
    for cap in caps:
        nch = -(-cap // 512)
        assert cap % nch == 0, (cap, nch)
        Ls.append(cap // nch)
        nchs.append(nch)
    goffs = [sum(caps[:r]) for r in range(NS)]
    C = sum(caps)
    # chunk list: (slot, chunk idx, global col offset, width)
    chunks = [
        (r, ci, goffs[r] + ci * Ls[r], Ls[r])
        for r in range(NS)
        for ci in range(nchs[r])
    ]
    order1 = [(r, f) for r in range(NS) for f in range(KFL)]

    nc = bacc.Bacc(None)
    # All our DMAs go through nc.sync (SP HWDGE). Dropping the unused queue
    # groups shrinks the NEFF's dma_queue_count (50 -> ~16); the end-of-kernel
    # per-queue completion-wait ladder on every engine shrinks with it.
    nc.m.queues = [q for q in nc.m.queues if q.name == "qSPDynamicHW"]
    x_ins = [
        nc.declare_dram_parameter(
            f"x{r}_pack", [nchs[r], P, KD, Ls[r]], mybir.dt.bfloat16, isOutput=False
        )
        for r in range(NS)
    ]
    # slot 0 w1: per-f-tile loads (head latency); slots 1-3: one merged load each
    w1s0_in = nc.declare_dram_parameter("w1_s0", [KFL, P, KD, P], mybir.dt.bfloat16, isOutput=False)
    w1r_ins = [
        nc.declare_dram_parameter(f"w1_s{r}", [P, KFL, KD, P], mybir.dt.bfloat16, isOutput=False)
        for r in range(1, NS)
    ]
    cf_in = nc.declare_dram_parameter("coef", [P, C], mybir.dt.bfloat16, isOutput=False)
    # w2 in 4 merged groups; group g holds (d=2g, 2g+1) x (r=0..3), i.e. dim1 = (d-2g)*NS+r
    w2_ins = [
        nc.declare_dram_parameter(f"w2_g{g}", [P, 2 * NS, KFL, P], mybir.dt.bfloat16, isOutput=False)
        for g in range(KD // 2)
    ]
    y_out = nc.declare_dram_parameter("y_dc", [KD, P, C], mybir.dt.bfloat16, isOutput=True)

    with tile.TileContext(nc) as tc:
        with (
            tc.tile_pool(name="wpool", bufs=1) as wpool,
            tc.tile_pool(name="apool", bufs=1) as apool,
            tc.tile_pool(name="ps", bufs=2, space="PSUM") as psp,
            tc.tile_pool(name="ypool", bufs=2) as ypool,
        ):
            # PE clock warm-up during the DMA head; borrows a ps1 buffer.
            warm = wpool.tile([P, P], mybir.dt.bfloat16, name="warm")
            nc.vector.memset(warm, 0.0)
            ps_w = psp.tile([P, Ls[0]], mybir.dt.float32, name=f"ps{Ls[0]}", tag=f"ps{Ls[0]}")
            for i in range(N_WARM):
                nc.tensor.matmul(
                    ps_w[:, :64], warm, warm[:, :64], start=(i == 0), stop=(i == N_WARM - 1)
                )

            w1_sb = {}

            def load_w1_s0(f):
                wt = wpool.tile([P, KD, P], mybir.dt.bfloat16, name=f"w1_0_{f}")
                nc.sync.dma_start(wt, w1s0_in[f])
                w1_sb[(0, f)] = wt

            def load_w1_slot(r):
                wt = wpool.tile([P, KFL, KD, P], mybir.dt.bfloat16, name=f"w1_r{r}")
                nc.sync.dma_start(wt, w1r_ins[r - 1][:])
                for f in range(KFL):
                    w1_sb[(r, f)] = wt[:, f]

            x_sb = {r: [None] * nchs[r] for r in range(NS)}

            def load_x(r):
                for ci in range(nchs[r]):
                    xt = apool.tile([P, KD, Ls[r]], mybir.dt.bfloat16, name=f"x_{r}_{ci}")
                    nc.sync.dma_start(xt, x_ins[r][ci])
                    x_sb[r][ci] = xt

            # loads in consumption order — the DMA rings are FIFO, so
            # late-enqueued bytes drain behind every earlier byte.
            load_w1_s0(0)
            load_x(0)
            for f in range(1, KFL):
                load_w1_s0(f)
            load_x(1)
            load_w1_slot(1)
            load_x(2)
            load_w1_slot(2)
            load_x(3)
            load_w1_slot(3)
            w2_sb = {}
            w2_tiles = []
            for g in range(KD // 2):
                wt = wpool.tile([P, 2 * NS, KFL, P], mybir.dt.bfloat16, name=f"w2_g{g}")
                nc.sync.dma_start(wt, w2_ins[g][:])
                w2_tiles.append(wt)
                for dd in range(2):
                    for r in range(NS):
                        w2_sb[(r, 2 * g + dd)] = wt[:, dd * NS + r]
                if g == 0:
                    coef_sb = apool.tile([P, C], mybir.dt.bfloat16, name="coef_sb")
                    nc.sync.dma_start(coef_sb, cf_in[:])

            h_sb = {
                r: apool.tile([P, KFL, caps[r]], mybir.dt.bfloat16, name=f"h_{r}")
                for r in range(NS)
            }

            # ---- stage 1: h_r[f] = gelu(w1_r[f] @ x_r.T)
            for r, f in order1:
                L = Ls[r]
                for ci in range(nchs[r]):
                    ps = psp.tile([P, L], mybir.dt.float32, name=f"ps{L}", tag=f"ps{L}")
                    for k in range(KD):
                        nc.tensor.matmul(
                            ps,
                            w1_sb[(r, f)][:, k],
                            x_sb[r][ci][:, k, :L],
                            start=(k == 0),
                            stop=(k == KD - 1),
                        )
                    c0 = ci * L
                    nc.scalar.activation(
                        out=h_sb[r][:, f, c0 : c0 + L],
                        in_=ps,
                        func=mybir.ActivationFunctionType.Gelu,
                    )

            # ---- stage 2: y_part.T = (w2_r @ h_r.T) * coef
            for d in range(KD):
                last_d = d == KD - 1
                y_sb = ypool.tile([P, C], mybir.dt.bfloat16, name="y_sb", tag="y_sb")
                # last iteration ends on the narrowest chunk so the final
                # mul+store after the last matmul group is minimal
                d_chunks = sorted(chunks, key=lambda c: -c[3]) if last_d else chunks
                for r, ci, g0, L in d_chunks:
                    c0 = ci * L
                    ps = psp.tile([P, L], mybir.dt.float32, name=f"ps{L}", tag=f"ps{L}")
                    for k2 in range(KFL):
                        nc.tensor.matmul(
                            ps,
                            w2_sb[(r, d)][:, k2],
                            h_sb[r][:, k2, c0 : c0 + L],
                            start=(k2 == 0),
                            stop=(k2 == KFL - 1),
                        )
                    nc.vector.tensor_mul(
                        y_sb[:, g0 : g0 + L], ps, coef_sb[:, g0 : g0 + L]
                    )
                    if last_d:
                        # per-chunk store so the post-matmul DMA tail is small
                        nc.sync.dma_start(y_out[d][:, g0 : g0 + L], y_sb[:, g0 : g0 + L])
                if not last_d:
                    nc.sync.dma_start(y_out[d], y_sb)
    nc.finalize()
    return nc


def _route(x: np.ndarray, gate_w: np.ndarray):
    logits = x.astype(np.float64) @ gate_w.astype(np.float64).T
    top2 = np.argsort(-logits, axis=1, kind="stable")[:, :2]
    v = np.take_along_axis(logits, top2, axis=1)
    v = v - v.max(axis=1, keepdims=True)
    ew = np.exp(v)
    w = ew / ew.sum(axis=1, keepdims=True)
    return top2, w.astype(np.float32)


def _pack_x(xe: np.ndarray, nch: int, L: int):
    """[D, C] fp32 -> [nch, P, KD, L] bf16."""
    return np.ascontiguousarray(
        xe.astype(BF16).reshape(KD, P, nch, L).transpose(2, 1, 0, 3)
    )


def _align_cap(c):
    """Smallest cap >= c that splits into equal <=512 chunks (even when >512)."""
    cap = int(c) + (int(c) & 1)
    while cap % -(-cap // 512):
        cap += 2
    return cap


def _plan(counts):
    """Snake-assign experts to 2 teams of 4; shared rank slot capacities."""
    aligned = [_align_cap(c) for c in counts]
    order = sorted(range(E), key=lambda e: -aligned[e])
    teams = [[], []]
    snake = [0, 1, 1, 0, 0, 1, 1, 0]
    for i, e in enumerate(order):
        teams[snake[i]].append(e)
    # teams[t] already in desc aligned order (order is desc)
    caps = tuple(
        max(aligned[teams[0][r]], aligned[teams[1][r]]) for r in range(NS)
    )
    return teams, caps


def _run(inputs: dict, trace: bool = False, trace_cores=None):
    x = np.asarray(inputs["x"], dtype=np.float32)
    gate_w = np.asarray(inputs["gate_w"], dtype=np.float32)
    w1 = np.asarray(inputs["w1"], dtype=np.float32)
    w2 = np.asarray(inputs["w2"], dtype=np.float32)
    n = x.shape[0]

    top2, wsm = _route(x, gate_w)

    idx_list, coef_list = [], []
    for e in range(E):
        mask = top2 == e
        idx = np.nonzero(mask.any(axis=1))[0]
        we = np.where(mask[idx, 0], wsm[idx, 0], wsm[idx, 1])
        idx_list.append(idx)
        coef_list.append(we.astype(np.float32))
    counts = [len(i) for i in idx_list]

    teams, caps = _plan(counts)
    goffs = [sum(caps[:r]) for r in range(NS)]
    C = sum(caps)
    nchs = [-(-cap // 512) for cap in caps]
    Ls = [cap // nch for cap, nch in zip(caps, nchs)]

    if caps not in _NC_CACHE:
        _NC_CACHE[caps] = _build_nc(caps)
    nc = _NC_CACHE[caps]

    in_maps = [None] * NCORE
    for t in range(2):
        texp = teams[t]  # 4 expert ids, slot order
        # per-slot x / coef shared by the team's 4 cores
        xpacks = {}
        coef = np.zeros((C,), dtype=np.float32)
        for r, e in enumerate(texp):
            xe = np.zeros((D, caps[r]), dtype=np.float32)
            xe[:, : counts[e]] = x[idx_list[e]].T
            xpacks[f"x{r}_pack"] = _pack_x(xe, nchs[r], Ls[r])
            coef[goffs[r] : goffs[r] + counts[e]] = coef_list[e]
        coef_rep = np.ascontiguousarray(np.broadcast_to(coef.astype(BF16), (P, C)))
        for s in range(NS):  # F-quarter index == core-within-team
            q = slice(s * FQ, (s + 1) * FQ)
            # per-slot w1 tiles [KFL, P, KD, P]: w1[e][q] is [FQ, D]: [f,c,k,p]->[f,p,k,c]
            w1_tiles = [
                w1[e][q].astype(BF16).reshape(KFL, P, KD, P).transpose(0, 3, 2, 1)
                for e in texp
            ]
            m = {"w1_s0": np.ascontiguousarray(w1_tiles[0])}
            for r in range(1, NS):
                # merged slot layout [P, KFL, KD, P]
                m[f"w1_s{r}"] = np.ascontiguousarray(w1_tiles[r].transpose(1, 0, 2, 3))
            # w2 tiles per (r): [KD, P, KFL, P]; group g = (d=2g,2g+1) x r -> [P, 2*NS, KFL, P]
            w2s = [
                w2[e][:, q].astype(BF16).reshape(KD, P, KFL, P).transpose(0, 3, 2, 1)
                for e in texp
            ]
            for g in range(KD // 2):
                grp = np.stack(
                    [w2s[r][2 * g + dd] for dd in range(2) for r in range(NS)]
                )  # [2*NS, P, KFL, P]
                m[f"w2_g{g}"] = np.ascontiguousarray(grp.transpose(1, 0, 2, 3))
            m["coef"] = coef_rep
            m.update(xpacks)
            in_maps[NS * t + s] = m

    res = run_bass_kernel_spmd(
        nc,
        in_maps,
        list(range(NCORE)),
        trace=trace,
        trace_cores=trace_cores,
    )

    out = np.zeros((n, D), dtype=np.float32)
    for t in range(2):
        ys = sum(
            np.asarray(res.results[NS * t + s]["y_dc"]).astype(np.float32)
            for s in range(NS)
        ).reshape(D, C)
        for r, e in enumerate(teams[t]):
            out[idx_list[e]] += ys[:, goffs[r] : goffs[r] + counts[e]].T
    return out, res


def kernel(**inputs) -> np.ndarray:
    out, _ = _run(inputs, trace=False)
    return out


if __name__ == "__main__":
    rng = np.random.default_rng(0)
    fake = {
        "x": rng.standard_normal((NTOK, D), dtype=np.float32),
        "gate_w": (rng.standard_normal((E, D)) * 0.02).astype(np.float32),
        "w1": (rng.standard_normal((E, F, D)) * 0.02).astype(np.float32),
        "w2": (rng.standard_normal((E, D, F)) * 0.02).astype(np.float32),
    }
    out = kernel(**fake)
    print("ok", out.shape, out.dtype, np.abs(out).max())
